# revision 22
# baseline (speedup 1.0000x reference)
"""Trainium2 Bass/Tile kernel for EntropyRecyclingLanguageNet (vq_codebook).

Computes, for x[B,D]:
    pw    = softmax(x @ attn_w + attn_b)          # [B,P]
    rec   = pw @ pattern_dict                      # [B,D]
    par   = rec @ self_w + self_b - rec            # [B,D]
    out   = (rec * sigmoid(||par||)) @ out_w + out_b   # [B,V]

Sharding: tensor-parallel over the vocab dim (V=32000 -> 4000 per core);
every core runs the full small stage for all B rows (cheap), and the
dominant cost -- writing the [8192, 4000] f32 output slice -- is spread
across the 8 cores.  Host gathers with a concat along axis 1.

Structure (per core):
  * x pre-transposed on host; logits computed TRANSPOSED in 512-wide
    blocks: logitsT[P, 512] = attn_w.T @ xT-block, with attn_b folded
    into the Exp activation bias (per-partition in this layout), giving
    expwT (unnormalized softmax numerators, transposed) directly.
  * denominators: per-tile PE transpose of expwT slices back to [B, P],
    with the PSUM->SBUF copy's fused accum_out giving rowsum(expw).
  * paradox*denom in ONE K=65 matmul per tile:
        [expwT; denom_row] @ [[pattern_dict @ (self_w - I)]; [self_b]]
    -- the recU intermediate and the rank-1 bias matmul are folded into
    the precomputed [P+1, D] right factor.  This path only feeds
    ||paradox|| -> sigmoid (a near-saturated scalar gate), so it runs in
    float32r single-pass PE mode.
  * sqrt/sigmoid batched once per group of 8 batch tiles so the ACT LUT
    reloads 3x per group instead of 3x per tile.
  * final projection folds softmax normalization, sigmoid gate and out_b
    into one K=65 float32r matmul per vocab tile:
        out = [sig/denom * expw, 1] @ [[pattern_dict @ out_w], [out_b]]
"""

import numpy as np

import concourse.bass as bass
import concourse.mybir as mybir
import concourse.tile as tile
from concourse import bacc
from concourse.bass_utils import run_bass_kernel_spmd

B, D, P, V = 8192, 128, 64, 32000
NCORES = 8
VS = V // NCORES        # vocab cols per core (4000)
VWIDTHS = [512] * 7 + [416]   # vocab tile widths (sum = VS)
BT = 128                # batch tile (partition dim)
NBT = B // BT           # 64 batch tiles
G = 8                   # batch tiles per transcendental group
W = 512                 # small-stage block width (4 batch tiles)
F32 = mybir.dt.float32
F32R = mybir.dt.float32r
AF = mybir.ActivationFunctionType

_cache = {}


def _build():
    nc = bacc.Bacc(
        "TRN2",
        target_bir_lowering=False,
        debug=False,
        num_devices=NCORES,
    )

    d_xT = nc.dram_tensor("xT", [D, B], F32, kind="ExternalInput").ap()
    d_attn_w = nc.dram_tensor("attn_w", [D, P], F32, kind="ExternalInput").ap()
    d_attn_b = nc.dram_tensor("attn_b", [P, 1], F32, kind="ExternalInput").ap()
    d_pdictT = nc.dram_tensor("pdictT", [D, P], F32, kind="ExternalInput").ap()
    d_swmi = nc.dram_tensor("swmi", [D, D], F32, kind="ExternalInput").ap()
    d_self_b = nc.dram_tensor("self_b", [1, D], F32, kind="ExternalInput").ap()
    d_ident = nc.dram_tensor("ident", [128, 128], F32, kind="ExternalInput").ap()
    d_ones64 = nc.dram_tensor("ones64", [P, 1], F32, kind="ExternalInput").ap()
    d_out_w = nc.dram_tensor("out_w", [D, VS], F32, kind="ExternalInput").ap()
    d_out_b = nc.dram_tensor("out_b", [1, VS], F32, kind="ExternalInput").ap()
    d_out = nc.dram_tensor("out", [B, VS], F32, kind="ExternalOutput").ap()

    with tile.TileContext(nc) as tc:
        with (
            tc.tile_pool(name="consts", bufs=1) as cpool,
            tc.tile_pool(name="expw", bufs=18) as epool,
            tc.tile_pool(name="wide", bufs=3) as wpool,
            tc.tile_pool(name="grp", bufs=2) as gpool,
            tc.tile_pool(name="small", bufs=3) as spool,
            tc.tile_pool(name="stage", bufs=3) as stpool,
            tc.tile_pool(name="pss", bufs=4, space="PSUM") as pss,
            tc.tile_pool(name="pso", bufs=4, space="PSUM") as pso,
        ):
            # ---- resident constants -------------------------------------
            # order matters: small consts + first x chunk first, so the
            # first compute block isn't queued behind the bulk loads
            attn_w = cpool.tile([D, P], F32)
            nc.sync.dma_start(attn_w[:], d_attn_w[:])
            attn_b = cpool.tile([P, 1], F32)
            nc.sync.dma_start(attn_b[:], d_attn_b[:])
            pdictT = cpool.tile([D, P], F32)
            nc.sync.dma_start(pdictT[:], d_pdictT[:])
            swmi = cpool.tile([D, D], F32)
            nc.sync.dma_start(swmi[:], d_swmi[:])
            self_b = cpool.tile([1, D], F32)
            nc.sync.dma_start(self_b[:], d_self_b[:])
            ident = cpool.tile([128, 128], F32)
            nc.sync.dma_start(ident[:], d_ident[:])
            ones64 = cpool.tile([P, 1], F32)
            nc.sync.dma_start(ones64[:], d_ones64[:])

            ident_r = cpool.tile([128, 128], F32R)
            nc.vector.tensor_copy(ident_r[:], ident[:])
            ones64_r = cpool.tile([P, 1], F32R)
            nc.vector.tensor_copy(ones64_r[:], ones64[:])
            attn_w_r = cpool.tile([D, P], F32R)
            nc.vector.tensor_copy(attn_w_r[:], attn_w[:])

            xT = cpool.tile([D, B], F32)
            xT_r = cpool.tile([D, B], F32R)
            out_w = cpool.tile([D, VS], F32)
            for c in range(8):  # chunked so batch tile 0 can start early
                nc.sync.dma_start(
                    xT[:, c * (B // 8):(c + 1) * (B // 8)],
                    d_xT[:, c * (B // 8):(c + 1) * (B // 8)],
                )
                nc.vector.tensor_copy(
                    xT_r[:, c * (B // 8):(c + 1) * (B // 8)],
                    xT[:, c * (B // 8):(c + 1) * (B // 8)],
                )
                if c == 0:
                    nc.sync.dma_start(out_w[:], d_out_w[:])

            # ---- m3 = [[pattern_dict @ (self_w - I)], [self_b]]  [P+1, D]
            m3 = cpool.tile([P + 1, D], F32R)
            ps_m3 = pss.tile([P, D], F32, tag="s", name="ps_m3")
            nc.tensor.matmul(ps_m3[:], pdictT[:], swmi[:], start=True, stop=True)
            nc.vector.tensor_copy(m3[0:P, :], ps_m3[:])
            nc.vector.tensor_copy(m3[P:P + 1, :], self_b[:])

            # ---- m2aug = [[pattern_dict @ out_w], [out_b]]  [P+1, VS] ----
            m2aug = cpool.tile([P + 1, VS], F32R)
            outb_stage = cpool.tile([1, VS], F32)
            nc.sync.dma_start(outb_stage[:], d_out_b[:])
            nc.vector.tensor_copy(m2aug[P:P + 1, :], outb_stage[:])
            off = 0
            for w in VWIDTHS:
                psm2 = pss.tile([P, w], F32, tag="s", name=f"psm2_{off}")
                nc.tensor.matmul(
                    psm2[:], pdictT[:], out_w[:, off:off + w],
                    start=True, stop=True,
                )
                nc.vector.tensor_copy(m2aug[0:P, off:off + w], psm2[:])
                off += w

            # ---- main loop: groups of G batch tiles ---------------------
            for g in range(NBT // G):
                dall = gpool.tile([BT, G], F32, tag="dall", name=f"dall_{g}")
                qall = gpool.tile([BT, G], F32, tag="qall", name=f"qall_{g}")
                expw_tiles = []

                # phase A: small stage in W-wide blocks (W//BT tiles each)
                for blk in range(G * BT // W):
                    i0 = g * G + blk * (W // BT)      # first tile of block
                    c0 = i0 * BT                       # batch col offset

                    ps_logT = pss.tile([P, W], F32, tag="s", name=f"ps_logT_{i0}")
                    nc.tensor.matmul(
                        ps_logT[:], attn_w_r[:], xT_r[:, c0:c0 + W],
                        start=True, stop=True,
                    )
                    # rows 0..P-1: expwT = exp(logitsT + attn_b);
                    # row P: the softmax denominators (filled below)
                    ewT = wpool.tile([P + 1, W], F32R, tag="ewT", name=f"ewT_{i0}")
                    nc.scalar.activation(ewT[0:P, :], ps_logT[:], AF.Exp, bias=attn_b[:])

                    ps_drow = pss.tile([1, W], F32, tag="s", name=f"ps_drow_{i0}")
                    nc.tensor.matmul(ps_drow[:], ones64_r[:], ewT[0:P, :], start=True, stop=True)
                    nc.vector.tensor_copy(ewT[P:P + 1, :], ps_drow[:])

                    for t in range(W // BT):
                        i = i0 + t
                        tg = i - g * G                 # index within group
                        sl = slice(t * BT, (t + 1) * BT)

                        # expw tile [B, P] back from the transposed form;
                        # the copy's accum gives the softmax denominator
                        ps_expw = pss.tile([BT, P], F32R, tag="s", name=f"ps_expw_{i}")
                        nc.tensor.transpose(ps_expw[:], ewT[0:P, sl], ident_r[0:P, 0:P])
                        expw = epool.tile([BT, P], F32, tag="expw", name=f"expw_{i}")
                        nc.scalar.activation(
                            expw[:], ps_expw[:], AF.Identity,
                            accum_out=dall[:, tg:tg + 1],
                        )
                        expw_tiles.append(expw)

                        # parScaled = recU@(self_w - I) + denom (x) self_b
                        # in one K=65 matmul against the fused m3 factor
                        ps_par = pss.tile([BT, D], F32, tag="s", name=f"ps_par_{i}")
                        nc.tensor.matmul(ps_par[:], ewT[:, sl], m3[:], start=True, stop=True)

                        sq = spool.tile([BT, D], F32, tag="sq", name=f"sq_{i}")
                        nc.scalar.activation(
                            sq[:], ps_par[:], AF.Square, accum_out=qall[:, tg:tg + 1]
                        )

                # group tail: sqrt/sigmoid once per group
                rdeng = gpool.tile([BT, G], F32, tag="rdeng", name=f"rdeng_{g}")
                nc.vector.reciprocal(rdeng[:], dall[:])
                nmagg = gpool.tile([BT, G], F32, tag="nmagg", name=f"nmagg_{g}")
                nc.scalar.activation(nmagg[:], qall[:], AF.Sqrt)
                magg = gpool.tile([BT, G], F32, tag="magg", name=f"magg_{g}")
                nc.vector.tensor_mul(magg[:], nmagg[:], rdeng[:])
                sigg = gpool.tile([BT, G], F32, tag="sigg", name=f"sigg_{g}")
                nc.scalar.activation(sigg[:], magg[:], AF.Sigmoid)
                sclg = gpool.tile([BT, G], F32, tag="sclg", name=f"sclg_{g}")
                nc.vector.tensor_mul(sclg[:], sigg[:], rdeng[:])

                # phase B: gated projection per tile
                for tg in range(G):
                    i = g * G + tg

                    pwa = spool.tile([BT, P + 1], F32, tag="pwa", name=f"pwa_{i}")
                    nc.vector.tensor_scalar_mul(
                        pwa[:, 0:P], expw_tiles[tg][:], sclg[:, tg:tg + 1]
                    )
                    nc.gpsimd.memset(pwa[:, P:P + 1], 1.0)
                    ps_pwT = pss.tile([P + 1, BT], F32, tag="s", name=f"ps_pwT_{i}")
                    nc.tensor.transpose(ps_pwT[:], pwa[:], ident[:])
                    pwaT = spool.tile([P + 1, BT], F32R, tag="pwaT", name=f"pwaT_{i}")
                    nc.vector.tensor_copy(pwaT[:], ps_pwT[:])

                    ob = stpool.tile([BT, VS], F32, tag="ob", name=f"ob_{i}")
                    off = 0
                    for jv, w in enumerate(VWIDTHS):
                        ps2 = pso.tile([BT, 512], F32, tag="o", name=f"ps2_{i}_{jv}")
                        nc.tensor.matmul(
                            ps2[:, 0:w], pwaT[:],
                            m2aug[:, off:off + w],
                            start=True, stop=True,
                        )
                        dst = ob[:, off:off + w]
                        if jv % 2 == 0:
                            nc.scalar.copy(dst, ps2[:, 0:w])
                        else:
                            nc.vector.tensor_copy(dst, ps2[:, 0:w])
                        off += w
                    nc.sync.dma_start(d_out[i * BT:(i + 1) * BT, :], ob[:])

    nc.compile()
    return nc


def _get_nc():
    if "nc" not in _cache:
        _cache["nc"] = _build()
    return _cache["nc"]


def make_in_maps(x, pattern_dict, attn_w, attn_b, self_w, self_b, out_w, out_b):
    x = np.ascontiguousarray(np.asarray(x, dtype=np.float32))
    pattern_dict = np.asarray(pattern_dict, dtype=np.float32)
    attn_w = np.asarray(attn_w, dtype=np.float32)
    attn_b = np.asarray(attn_b, dtype=np.float32)
    self_w = np.asarray(self_w, dtype=np.float32)
    self_b = np.asarray(self_b, dtype=np.float32)
    out_w = np.asarray(out_w, dtype=np.float32)
    out_b = np.asarray(out_b, dtype=np.float32)

    shared = {
        "xT": np.ascontiguousarray(x.T),
        "attn_w": np.ascontiguousarray(attn_w),
        "attn_b": np.ascontiguousarray(attn_b.reshape(P, 1)),
        "pdictT": np.ascontiguousarray(pattern_dict.T),
        "swmi": np.ascontiguousarray(self_w - np.eye(D, dtype=np.float32)),
        "self_b": np.ascontiguousarray(self_b.reshape(1, D)),
        "ident": np.eye(128, dtype=np.float32),
        "ones64": np.ones((P, 1), dtype=np.float32),
    }
    in_maps = []
    for c in range(NCORES):
        m = dict(shared)
        m["out_w"] = np.ascontiguousarray(out_w[:, c * VS:(c + 1) * VS])
        m["out_b"] = np.ascontiguousarray(out_b[c * VS:(c + 1) * VS].reshape(1, VS))
        in_maps.append(m)
    return in_maps


def kernel(x, pattern_dict, attn_w, attn_b, self_w, self_b, out_w, out_b):
    in_maps = make_in_maps(
        x, pattern_dict, attn_w, attn_b, self_w, self_b, out_w, out_b
    )
    nc = _get_nc()
    res = run_bass_kernel_spmd(nc, in_maps, list(range(NCORES)))
    return np.concatenate([res.results[c]["out"] for c in range(NCORES)], axis=1)
